# revision 1
# baseline (speedup 1.0000x reference)
"""Two-layer GCN (PyG GCNConv x2 + rrelu) on 8 Trainium2 NeuronCores.

Math: with A = adjacency-with-multiplicity + I (self loops), deg = in-degree
(including the self loop), dinv = deg^-1/2:
    z1[v] = dinv[v] * (sum_{u->v} dinv[u]*x[u]) @ W1 + b1
    g[u]  = dinv[u] * rrelu(z1[u])                      (dinv pre-folded for L2)
    z2[v] = dinv[v] * (sum_{u->v} g[u]) @ W2 + b2
Aggregation is linear, so the dense W matmul is applied post-aggregation on
the [128, 128] per-destination-block aggregate -- 128x less PE work than
transforming every edge message.

Sharding: destinations are range-sharded across the 8 cores (12544 each).
Every core keeps a replicated (dinv-prescaled, bf16) source-feature table in
its own HBM and fetches the source rows of its edges with dma_gather (int16
indices -> four even source windows; one call per (block, window), capped at
<=1008 indices by the 64-descriptor/engine SWDGE ring).  Per destination
block of 128 nodes, gathered edge-message chunks [128 edges, 128 feat] are
scatter-reduced on the TensorEngine by matmul with one-hot selectors
Sel[e, dest] = (d[e] == dest) generated on-device (is_equal with broadcast
operand).  Self-loop contributions bypass the gather: their source rows are
contiguous, so a plain DMA + identity matmul adds them.  Two NEFF dispatches
(layer 1, layer 2); the host transposes/concats activations between them.

The harness calls kernel(**inputs) with full inputs; index bucketing,
program build, compile, SPMD run on cores 0-7 and unshard all happen here.
"""

import sys

for _p in ("/opt/trn_rl_repo",):
    if _p not in sys.path:
        sys.path.insert(0, _p)

import numpy as np
import ml_dtypes

import concourse.bacc as bacc
import concourse.bass as bass
import concourse.mybir as mybir
import concourse.tile as tile
from concourse.bass_utils import run_bass_kernel_spmd

P = 128  # partition width == dest block width == feature width
RRELU_SLOPE = (1.0 / 8.0 + 1.0 / 3.0) / 2.0
MAX_CALL_COLS = 7   # dma_gather is capped at 1008 indices per call


class Cfg:
    def __init__(self, n_nodes, n_cores, blocks_per_core, superblock, in_f,
                 out1_f, out2_f, src_window, min_cap=1):
        self.n_nodes = n_nodes
        self.n_cores = n_cores
        self.bpc = blocks_per_core            # dest blocks per core
        self.sb = superblock                  # blocks per superblock
        assert blocks_per_core % superblock == 0
        self.sb_count = blocks_per_core // superblock
        self.in_f = in_f
        self.out1_f = out1_f
        self.out2_f = out2_f
        self.src_window = src_window          # int16 gather range per window
        self.min_cap = min_cap
        self.nodes_per_core = blocks_per_core * P
        self.n_pad = n_cores * self.nodes_per_core
        assert self.n_pad >= n_nodes
        assert src_window % P == 0 and src_window <= 32768
        self.n_chunks = -(-self.n_pad // src_window)
        self.tab_rows = self.n_chunks * src_window


FULL = Cfg(n_nodes=100000, n_cores=8, blocks_per_core=98, superblock=7,
           in_f=128, out1_f=128, out2_f=64, src_window=25088, min_cap=5)


def _call_plan(caps):
    """Per-block gather calls: (window k, col offset, n_cols), <=7 cols each."""
    plan = []
    for k, cap in enumerate(caps):
        c0 = 0
        while c0 < cap:
            n = min(MAX_CALL_COLS, cap - c0)
            plan.append((k, c0, n))
            c0 += n
    return plan


# --------------------------------------------------------------------------
# host-side index preprocessing
# --------------------------------------------------------------------------

def preprocess(edge_index, cfg):
    """Bucket edges by (dest block, src window); self loops are handled
    separately on-device.  Build per-core gather index / dest-local tables
    and the degree scaling."""
    row = edge_index[0].astype(np.int64)
    col = edge_index[1].astype(np.int64)
    n = cfg.n_nodes

    deg = np.bincount(col, minlength=cfg.n_pad).astype(np.float64) + 1.0
    dinv = (1.0 / np.sqrt(deg)).astype(np.float32)
    dinv[n:] = 1.0

    blk = col >> 7                      # global dest block
    chunk = row // cfg.src_window
    order = np.lexsort((chunk, blk))
    row, col, blk, chunk = row[order], col[order], blk[order], chunk[order]

    n_blocks = cfg.n_cores * cfg.bpc
    counts = np.zeros((n_blocks, cfg.n_chunks), dtype=np.int64)
    np.add.at(counts, (blk, chunk), 1)

    caps = np.maximum(-(-counts.max(axis=0) // P), cfg.min_cap).astype(np.int64)
    c_total = int(caps.sum())
    colbase = np.concatenate([[0], np.cumsum(caps)])[:-1]

    bc_start = np.zeros(n_blocks * cfg.n_chunks + 1, dtype=np.int64)
    np.cumsum(counts.reshape(-1), out=bc_start[1:])

    plan = _call_plan([int(x) for x in caps])
    per_core = []
    for c in range(cfg.n_cores):
        idx_parts = []
        d_tab = np.full((P, cfg.bpc * c_total), -1.0, dtype=np.float64)
        for b_loc in range(cfg.bpc):
            b_glob = c * cfg.bpc + b_loc
            segs = []
            for k in range(cfg.n_chunks):
                cap = int(caps[k])
                lo = bc_start[b_glob * cfg.n_chunks + k]
                hi = bc_start[b_glob * cfg.n_chunks + k + 1]
                cnt = hi - lo
                assert cnt <= cap * P, (cnt, cap * P)
                seg = np.zeros(cap * P, dtype=np.int64)
                seg[:cnt] = row[lo:hi] - k * cfg.src_window
                if cnt < cap * P:             # duplicate-pad (d stays -1)
                    seg[cnt:] = seg[0] if cnt > 0 else 0
                assert seg.min() >= 0 and seg.max() < cfg.src_window
                segs.append(seg)
                gcol0 = b_loc * c_total + colbase[k]
                d_seg = np.full(cap * P, -1.0)
                d_seg[:cnt] = (col[lo:hi] - b_glob * P).astype(np.float64)
                d_tab[:, gcol0:gcol0 + cap] = d_seg.reshape(cap, P).T
            for (k, c0, ncols) in plan:
                idx_parts.append(segs[k][c0 * P:(c0 + ncols) * P].astype(np.int16))
        idx_flat = [a.reshape(-1, 16).T for a in idx_parts]
        idx_tab = np.concatenate(idx_flat, axis=1)
        idx_tab = np.tile(idx_tab, (8, 1))          # [128, total/16]
        per_core.append({
            "idx_tab": np.ascontiguousarray(idx_tab),
            "d_tab": np.ascontiguousarray(d_tab.astype(ml_dtypes.bfloat16)),
            "dinv_sl": np.ascontiguousarray(
                dinv[c * cfg.nodes_per_core:(c + 1) * cfg.nodes_per_core]
            ).reshape(1, -1),
        })

    return {"caps": caps, "c_total": c_total, "dinv": dinv,
            "per_core": per_core}


# --------------------------------------------------------------------------
# bass program (one GCN layer, SPMD across cores; all data via inputs)
# --------------------------------------------------------------------------

def build_layer_program(cfg, caps, layer):
    """layer=1: out = bf16 gs1T [128, nodes_per_core]  (dinv*rrelu(z1), F-major)
       layer=2: out = f32  z2T  [out2_f, nodes_per_core]"""
    caps = [int(x) for x in caps]
    c_total = sum(caps)
    plan = _call_plan(caps)
    out_f = cfg.out1_f if layer == 1 else cfg.out2_f
    out_dt = mybir.dt.bfloat16 if layer == 1 else mybir.dt.float32
    idx_cols_blk = c_total * P // 16         # idx free-dim per block
    G = 8                                     # sel-gen chunk group width

    nc = bacc.Bacc("TRN2", target_bir_lowering=False, debug=False,
                   num_devices=cfg.n_cores,
                   num_swdge_queues=min(4, cfg.n_chunks))
    dt = mybir.dt
    src_tab = nc.dram_tensor("src_tab", [cfg.tab_rows, P], dt.bfloat16,
                             kind="ExternalInput")
    w_in = nc.dram_tensor("w", [P, out_f], dt.bfloat16, kind="ExternalInput")
    bias_in = nc.dram_tensor("bias", [out_f, 1], dt.float32, kind="ExternalInput")
    dinv_in = nc.dram_tensor("dinv_sl", [1, cfg.nodes_per_core], dt.float32,
                             kind="ExternalInput")
    idx_in = nc.dram_tensor("idx_tab", [P, cfg.bpc * idx_cols_blk], dt.int16,
                            kind="ExternalInput")
    d_in = nc.dram_tensor("d_tab", [P, cfg.bpc * c_total], dt.bfloat16,
                          kind="ExternalInput")
    iota_in = nc.dram_tensor("iota", [P, G * P], dt.bfloat16, kind="ExternalInput")
    ident_in = nc.dram_tensor("ident", [P, P], dt.bfloat16, kind="ExternalInput")
    ones_in = nc.dram_tensor("ones", [1, P], dt.float32, kind="ExternalInput")
    out_t = nc.dram_tensor("out_t", [out_f, cfg.nodes_per_core], out_dt,
                           kind="ExternalOutput")
    # per-core self-loop source rows, staged by the host (node-major slice of
    # src_tab rows owned by this core; avoids needing the core id on device)
    self_in = nc.dram_tensor("self_rows", [cfg.nodes_per_core, P], dt.bfloat16,
                             kind="ExternalInput")

    with tile.TileContext(nc) as tc:
        with (
            tc.tile_pool(name="const", bufs=1) as const_pool,
            tc.tile_pool(name="idx", bufs=2) as idx_pool,
            tc.tile_pool(name="msg", bufs=2) as msg_pool,
            tc.tile_pool(name="selfp", bufs=2) as self_pool,
            tc.tile_pool(name="sel", bufs=6) as sel_pool,
            tc.tile_pool(name="aggsb", bufs=3) as aggsb_pool,
            tc.tile_pool(name="tmp", bufs=3) as tmp_pool,
            tc.tile_pool(name="outsb", bufs=2) as out_pool,
            tc.tile_pool(name="psA", bufs=2, space="PSUM") as agg_psum,
            tc.tile_pool(name="psZ", bufs=2, space="PSUM") as z_psum,
            tc.tile_pool(name="psD", bufs=2, space="PSUM") as d_psum,
        ):
            w_sb = const_pool.tile([P, out_f], dt.bfloat16)
            nc.sync.dma_start(out=w_sb[:], in_=w_in[:])
            bias_sb = const_pool.tile([out_f, 1], dt.float32)
            nc.sync.dma_start(out=bias_sb[:], in_=bias_in[:])
            dinv_sb = const_pool.tile([1, cfg.nodes_per_core], dt.float32)
            nc.sync.dma_start(out=dinv_sb[:], in_=dinv_in[:])
            iota_sb = const_pool.tile([P, G * P], dt.bfloat16)
            nc.sync.dma_start(out=iota_sb[:], in_=iota_in[:])
            ident_sb = const_pool.tile([P, P], dt.bfloat16)
            nc.sync.dma_start(out=ident_sb[:], in_=ident_in[:])
            ones_sb = const_pool.tile([1, P], dt.float32)
            nc.sync.dma_start(out=ones_sb[:], in_=ones_in[:])
            d_sb = const_pool.tile([P, cfg.bpc * c_total], dt.bfloat16)
            nc.sync.dma_start(out=d_sb[:], in_=d_in[:])

            self_view = self_in.rearrange("(s b p) f -> s p b f",
                                          p=P, b=cfg.sb)

            for s in range(cfg.sb_count):
                idx_sb = idx_pool.tile([P, cfg.sb * idx_cols_blk], dt.int16)
                nc.sync.dma_start(
                    out=idx_sb[:],
                    in_=idx_in[:, s * cfg.sb * idx_cols_blk:
                               (s + 1) * cfg.sb * idx_cols_blk])
                # contiguous self-loop rows for this superblock
                selfs = self_pool.tile([P, cfg.sb, P], dt.bfloat16)
                nc.sync.dma_start(out=selfs[:], in_=self_view[s])

                msg = msg_pool.tile([P, cfg.sb * c_total, P], dt.bfloat16)
                off = 0
                for b7 in range(cfg.sb):
                    for (k, c0, ncols) in plan:
                        n_idx = ncols * P
                        mcol0 = b7 * c_total + colbase_val(caps, k) + c0
                        nc.gpsimd.dma_gather(
                            msg[:, mcol0:mcol0 + ncols, :],
                            src_tab[k * cfg.src_window:
                                    (k + 1) * cfg.src_window, :],
                            idx_sb[:, off:off + n_idx // 16],
                            n_idx, n_idx, P,
                            queue_num=k % 4,
                        )
                        off += n_idx // 16

                out_sb = out_pool.tile([out_f, cfg.sb * P], out_dt)
                for b7 in range(cfg.sb):
                    b_loc = s * cfg.sb + b7
                    dcol0 = b_loc * c_total
                    sels = []
                    done = 0
                    while done < c_total:
                        g = min(G, c_total - done)
                        sel = sel_pool.tile([P, G * P], dt.bfloat16)
                        nc.vector.tensor_tensor(
                            sel[:, :g * P],
                            iota_sb[:, :g * P],
                            d_sb[:, dcol0 + done:dcol0 + done + g]
                                .to_broadcast([P, g, P]),
                            mybir.AluOpType.is_equal,
                        )
                        sels.extend((sel, j) for j in range(g))
                        done += g

                    agg = agg_psum.tile([P, P], dt.float32)
                    for ci, (sel, j) in enumerate(sels):
                        nc.tensor.matmul(
                            agg[:],
                            lhsT=msg[:, b7 * c_total + ci, :],
                            rhs=sel[:, j * P:(j + 1) * P],
                            start=(ci == 0), stop=False,
                        )
                    # self-loop contribution: aggT += selfs[:, b7, :]^T
                    nc.tensor.matmul(
                        agg[:], lhsT=selfs[:, b7, :], rhs=ident_sb[:],
                        start=False, stop=True)

                    # dinv broadcast tile for this block (rank-1 matmul into
                    # psum, then to SBUF via the idle ScalarEngine -- DVE may
                    # read only one PSUM operand and zps is already PSUM)
                    dps = d_psum.tile([P, P], dt.float32)
                    nc.tensor.matmul(
                        dps[:], lhsT=ones_sb[:],
                        rhs=dinv_sb[:, b_loc * P:(b_loc + 1) * P],
                        start=True, stop=True)
                    dbc = aggsb_pool.tile([P, P], dt.float32, tag="dbc")
                    nc.scalar.copy(dbc[:], dps[:])

                    aggsb = aggsb_pool.tile([P, P], dt.bfloat16, tag="aggsb")
                    nc.vector.tensor_copy(aggsb[:], agg[:])

                    zps = z_psum.tile([out_f, P], dt.float32)
                    nc.tensor.matmul(zps[:], lhsT=w_sb[:], rhs=aggsb[:],
                                     start=True, stop=True)

                    o_sl = out_sb[:, b7 * P:(b7 + 1) * P]
                    bias_bc = bias_sb[:, 0:1].to_broadcast([out_f, P])
                    if layer == 1:
                        t1 = tmp_pool.tile([P, P], dt.float32, tag="t1")
                        nc.vector.tensor_tensor(t1[:], zps[:], dbc[:],
                                                mybir.AluOpType.mult)
                        u = tmp_pool.tile([P, P], dt.float32, tag="u")
                        nc.vector.tensor_tensor(u[:], t1[:], bias_bc,
                                                mybir.AluOpType.add)
                        rr = tmp_pool.tile([P, P], dt.float32, tag="rr")
                        nc.vector.scalar_tensor_tensor(
                            rr[:], u[:], float(RRELU_SLOPE), u[:],
                            mybir.AluOpType.mult, mybir.AluOpType.max)
                        nc.vector.tensor_tensor(o_sl, rr[:], dbc[:],
                                                mybir.AluOpType.mult)
                    else:
                        t1 = tmp_pool.tile([out_f, P], dt.float32, tag="t1")
                        nc.vector.tensor_tensor(t1[:], zps[:], dbc[:out_f, :],
                                                mybir.AluOpType.mult)
                        nc.vector.tensor_tensor(o_sl, t1[:], bias_bc,
                                                mybir.AluOpType.add)

                nc.sync.dma_start(
                    out=out_t[:, s * cfg.sb * P:(s + 1) * cfg.sb * P],
                    in_=out_sb[:])

    nc.compile()
    return nc


def colbase_val(caps, k):
    return int(sum(caps[:k]))


# --------------------------------------------------------------------------
# orchestration
# --------------------------------------------------------------------------

def _iota_tile(G=8):
    return np.tile(np.arange(P, dtype=np.float32), G)[None, :].repeat(P, 0).astype(ml_dtypes.bfloat16)


def _run_gcn(x, edge_index, W1, b1, W2, b2, cfg, runner=None, want_times=False):
    """Shared driver; runner(nc, in_maps) -> list of per-core output dicts."""
    meta = preprocess(np.asarray(edge_index), cfg)
    dinv = meta["dinv"]
    npc = cfg.nodes_per_core

    if runner is None:
        times = []

        def runner(nc, in_maps):
            r = run_bass_kernel_spmd(nc, in_maps, core_ids=list(range(cfg.n_cores)),
                                     trace=want_times)
            if want_times:
                times.append(r.exec_time_ns)
            return r.results
    else:
        times = None

    x = np.asarray(x, dtype=np.float32)
    xs = np.zeros((cfg.tab_rows, P), dtype=ml_dtypes.bfloat16)
    xs[:cfg.n_nodes] = (x * dinv[:cfg.n_nodes, None]).astype(ml_dtypes.bfloat16)

    iota = _iota_tile()
    ident = np.eye(P, dtype=np.float32).astype(ml_dtypes.bfloat16)
    ones = np.ones((1, P), np.float32)
    w1 = np.asarray(W1, np.float32).astype(ml_dtypes.bfloat16)
    w2 = np.asarray(W2, np.float32).astype(ml_dtypes.bfloat16)
    b1c = np.asarray(b1, np.float32).reshape(-1, 1)
    b2c = np.asarray(b2, np.float32).reshape(-1, 1)

    nc1 = build_layer_program(cfg, meta["caps"], layer=1)
    in_maps = [
        {"src_tab": xs, "w": w1, "bias": b1c, "iota": iota, "ident": ident,
         "ones": ones,
         "self_rows": np.ascontiguousarray(xs[c * npc:(c + 1) * npc]),
         **{k: pc[k] for k in ("idx_tab", "d_tab", "dinv_sl")}}
        for c, pc in enumerate(meta["per_core"])
    ]
    res1 = runner(nc1, in_maps)

    gs = np.zeros((cfg.tab_rows, P), dtype=ml_dtypes.bfloat16)
    for c in range(cfg.n_cores):
        gs[c * npc:(c + 1) * npc] = res1[c]["out_t"].T

    nc2 = build_layer_program(cfg, meta["caps"], layer=2)
    for c in range(cfg.n_cores):
        in_maps[c] = dict(in_maps[c])
        in_maps[c]["src_tab"] = gs
        in_maps[c]["self_rows"] = np.ascontiguousarray(gs[c * npc:(c + 1) * npc])
        in_maps[c]["w"] = w2
        in_maps[c]["bias"] = b2c
    res2 = runner(nc2, in_maps)

    out = np.zeros((cfg.n_pad, cfg.out2_f), dtype=np.float32)
    for c in range(cfg.n_cores):
        out[c * npc:(c + 1) * npc] = res2[c]["out_t"].T
    out = out[:cfg.n_nodes]
    if want_times and times is not None:
        return out, times
    return out


def kernel(x, edge_index, W1, b1, W2, b2):
    return _run_gcn(x, edge_index, W1, b1, W2, b2, FULL)



# revision 2
# speedup vs baseline: 1.1153x; 1.1153x over previous
"""Two-layer GCN (PyG GCNConv x2 + rrelu) on 8 Trainium2 NeuronCores.

Math: with A = adjacency-with-multiplicity + I (self loops), deg = in-degree
(including the self loop), dinv = deg^-1/2:
    z1[v] = dinv[v] * (sum_{u->v} dinv[u]*x[u]) @ W1 + b1
    g[u]  = dinv[u] * rrelu(z1[u])                      (dinv pre-folded for L2)
    z2[v] = dinv[v] * (sum_{u->v} g[u]) @ W2 + b2
Aggregation is linear, so the dense W matmul is applied post-aggregation on
the [128, 128] per-destination-block aggregate.

Sharding: destinations range-sharded across 8 cores (12544 each).  Every core
keeps a replicated (dinv-prescaled, bf16) source-feature table in HBM and
fetches the source rows of its edges with dma_gather.  Gathers are issued as
ONE large call per (superblock, source-window) (~4K indices each; the SWDGE
ring holds 1024 descriptors/engine so this is far under the cap), which
amortizes the ~1us fixed SWDGE cost that dominated the old per-(block,window)
call scheme.  Edge slots are packed back-to-back with per-(block,window)
segment lengths fixed to the max across cores (SPMD uniformity) -- no
128-padding per bucket.  Self-loop rows are staged host-side in a
partition-major layout and land in the message tile via one fat DMA per
superblock.  Scatter onto destinations is a TensorE matmul with one-hot
selectors generated on DVE (one is_equal per destination block).
"""

import sys

for _p in ("/opt/trn_rl_repo",):
    if _p not in sys.path:
        sys.path.insert(0, _p)

import numpy as np
import ml_dtypes

import concourse.bacc as bacc
import concourse.bass as bass
import concourse.mybir as mybir
import concourse.tile as tile
from concourse.bass_utils import run_bass_kernel_spmd

P = 128
RRELU_SLOPE = (1.0 / 8.0 + 1.0 / 3.0) / 2.0


class Cfg:
    def __init__(self, n_nodes, n_cores, blocks_per_core, superblock, in_f,
                 out1_f, out2_f, src_window):
        self.n_nodes = n_nodes
        self.n_cores = n_cores
        self.bpc = blocks_per_core
        self.sb = superblock
        assert blocks_per_core % superblock == 0
        self.sb_count = blocks_per_core // superblock
        self.in_f = in_f
        self.out1_f = out1_f
        self.out2_f = out2_f
        self.src_window = src_window
        self.nodes_per_core = blocks_per_core * P
        self.n_pad = n_cores * self.nodes_per_core
        assert self.n_pad >= n_nodes
        self.n_chunks = -(-self.n_pad // src_window)
        self.tab_rows = self.n_chunks * src_window


FULL = Cfg(n_nodes=100000, n_cores=8, blocks_per_core=98, superblock=7,
           in_f=128, out1_f=128, out2_f=64, src_window=25088)

MAX_CALL_IDX = 8064          # safety split (ring allows ~16K)


def _ru(x, m):
    return -(-x // m) * m


# --------------------------------------------------------------------------
# host-side index preprocessing
# --------------------------------------------------------------------------

def preprocess(edge_index, cfg):
    """Bucket edges by (core, superblock, src window, block); build the
    uniform (cross-core) segment structure, the shared matmul schedule, and
    per-core idx / d tables."""
    row = edge_index[0].astype(np.int64)
    col = edge_index[1].astype(np.int64)
    n = cfg.n_nodes
    npc = cfg.nodes_per_core
    NSB, NK, SBW = cfg.sb_count, cfg.n_chunks, cfg.sb

    deg = np.bincount(col, minlength=cfg.n_pad).astype(np.float64) + 1.0
    dinv = (1.0 / np.sqrt(deg)).astype(np.float32)
    dinv[n:] = 1.0

    core = col // npc
    col_loc = col % npc
    blk = col_loc >> 7                    # block within core (0..97)
    s = blk // SBW                        # superblock
    b7 = blk % SBW
    k = row // cfg.src_window
    dloc = col_loc & 127

    # counts per (core, s, k, b7)
    cnt = np.zeros((cfg.n_cores, NSB, NK, SBW), dtype=np.int64)
    np.add.at(cnt, (core, s, k, b7), 1)
    seg_len = cnt.max(axis=0)             # [NSB, NK, SBW] uniform

    # section/column structure (shared across cores)
    seg_start = np.zeros_like(seg_len)    # slot offset within (s, k) section
    sec_tot = np.zeros((NSB, NK), dtype=np.int64)
    sec_pad = np.zeros((NSB, NK), dtype=np.int64)   # roundup128
    for si in range(NSB):
        for ki in range(NK):
            c0 = 0
            for b in range(SBW):
                seg_start[si, ki, b] = c0
                c0 += seg_len[si, ki, b]
            sec_tot[si, ki] = c0
            sec_pad[si, ki] = _ru(max(c0, 1), P)

    # msg tile columns: cols 0..6 = self rows (block b7 <-> col b7);
    # then per k section, sec_pad/128 columns.
    sec_col0 = np.zeros((NSB, NK), dtype=np.int64)
    msg_cols = np.zeros(NSB, dtype=np.int64)
    for si in range(NSB):
        c = SBW
        for ki in range(NK):
            sec_col0[si, ki] = c
            c += sec_pad[si, ki] // P
        msg_cols[si] = c

    # idx tile columns per superblock (int16, 16-wrapped)
    idx_col0 = np.zeros((NSB, NK), dtype=np.int64)
    idx_cols = np.zeros(NSB, dtype=np.int64)
    for si in range(NSB):
        c = 0
        for ki in range(NK):
            idx_col0[si, ki] = c
            c += sec_pad[si, ki] // 16
        idx_cols[si] = c
    idx_off = np.concatenate([[0], np.cumsum(idx_cols)])   # per-sb offset
    ICOLS = int(idx_off[-1])

    # matmul schedule + d-column ids, per (s, b7):
    #   entries: list of (msg_col, dcol_within_block)
    #   dcol 0 is always the self column (d = identity ramp)
    mm_sched = [[None] * SBW for _ in range(NSB)]
    ndcols = np.zeros((NSB, SBW), dtype=np.int64)
    # map (s, k, msg_col, b7) -> dcol id, for host d-table fill
    dcol_of = {}
    dcol_base = np.zeros((NSB, SBW), dtype=np.int64)
    DCOLS = 0
    for si in range(NSB):
        for b in range(SBW):
            ents = [(b, 0)]                       # self column
            nd = 1
            for ki in range(NK):
                st = int(seg_start[si, ki, b])
                ln = int(seg_len[si, ki, b])
                if ln == 0:
                    continue
                c0, c1 = st // P, (st + ln - 1) // P
                for cc in range(c0, c1 + 1):
                    mcol = int(sec_col0[si, ki]) + cc
                    ents.append((mcol, nd))
                    dcol_of[(si, ki, cc, b)] = nd
                    nd += 1
            mm_sched[si][b] = ents
            ndcols[si, b] = nd
            dcol_base[si, b] = DCOLS
            DCOLS += nd
    G_MAX = int(ndcols.max())

    # ---- per-core tables (vectorized over edges) -------------------------
    # rank of each edge within its (core, s, k, b7) group
    gid = ((core * NSB + s) * NK + k) * SBW + b7
    order = np.argsort(gid, kind="stable")
    gsort = gid[order]
    grp_start = np.zeros(cfg.n_cores * NSB * NK * SBW + 1, dtype=np.int64)
    np.cumsum(np.bincount(gsort, minlength=grp_start.size - 1), out=grp_start[1:])
    rank = np.empty(row.size, dtype=np.int64)
    rank[order] = np.arange(row.size) - grp_start[gsort]

    slot_in_sec = seg_start[s, k, b7] + rank          # slot within (s,k)
    sec_colv = slot_in_sec >> 7                       # col within section
    sec_p = slot_in_sec & 127                         # partition
    idx_val = (row - k * cfg.src_window).astype(np.int16)

    # idx flat position: (16-wrap) idx_tab[16*rep + (slot%16), off + slot//16]
    idx_colv = idx_off[s] + idx_col0[s, k] + (slot_in_sec >> 4)
    idx_rowv = slot_in_sec & 15

    # d position: [p, dcol_base[s,b7] + dcol_of[s,k,col,b7]]
    dcol_l = np.empty(row.size, dtype=np.int64)
    # build lookup array for dcol_of: key (s,k,col,b7) -> id
    max_cols = int((sec_pad // P).max())
    dlk = np.full((NSB, NK, max_cols, SBW), -1, dtype=np.int64)
    for (si, ki, cc, b), v in dcol_of.items():
        dlk[si, ki, cc, b] = v
    dcol_l = dlk[s, k, sec_colv, b7]
    assert (dcol_l >= 0).all()
    d_colv = dcol_base[s, b7] + dcol_l

    per_core = []
    for c in range(cfg.n_cores):
        m = core == c
        it = np.zeros((16, ICOLS), dtype=np.int16)
        it[idx_rowv[m], idx_colv[m]] = idx_val[m]
        idx_tab = np.tile(it, (8, 1))
        d_tab = np.full((P, DCOLS), -1.0, dtype=np.float64)
        d_tab[sec_p[m], d_colv[m]] = dloc[m].astype(np.float64)
        # self d-columns: identity ramp
        for si in range(NSB):
            for b in range(SBW):
                d_tab[:, dcol_base[si, b]] = np.arange(P)
        per_core.append({
            "idx_tab": np.ascontiguousarray(idx_tab),
            "d_tab": np.ascontiguousarray(d_tab.astype(ml_dtypes.bfloat16)),
            "dinv_sl": np.ascontiguousarray(
                dinv[c * npc:(c + 1) * npc]).reshape(1, -1),
        })

    shared = {
        "sec_pad": sec_pad, "sec_col0": sec_col0, "msg_cols": msg_cols,
        "idx_col0": idx_col0, "idx_cols": idx_cols, "idx_off": idx_off,
        "ICOLS": ICOLS, "DCOLS": DCOLS, "G_MAX": G_MAX,
        "mm_sched": mm_sched, "ndcols": ndcols, "dcol_base": dcol_base,
    }
    return {"dinv": dinv, "per_core": per_core, "shared": shared}


# --------------------------------------------------------------------------
# bass program (one GCN layer, SPMD across cores)
# --------------------------------------------------------------------------

def build_layer_program(cfg, shared, layer):
    NSB, NK, SBW = cfg.sb_count, cfg.n_chunks, cfg.sb
    out_f = cfg.out1_f if layer == 1 else cfg.out2_f
    out_dt = mybir.dt.bfloat16 if layer == 1 else mybir.dt.float32
    ICOLS, DCOLS, G_MAX = shared["ICOLS"], shared["DCOLS"], shared["G_MAX"]
    sec_pad, sec_col0 = shared["sec_pad"], shared["sec_col0"]
    msg_cols = shared["msg_cols"]
    idx_col0, idx_cols, idx_off = (shared["idx_col0"], shared["idx_cols"],
                                   shared["idx_off"])
    mm_sched, ndcols, dcol_base = (shared["mm_sched"], shared["ndcols"],
                                   shared["dcol_base"])
    MSG_MAX = int(msg_cols.max())
    IDX_MAX = int(idx_cols.max())

    nc = bacc.Bacc("TRN2", target_bir_lowering=False, debug=False,
                   num_devices=cfg.n_cores, num_swdge_queues=4)
    dt = mybir.dt
    src_tab = nc.dram_tensor("src_tab", [cfg.tab_rows, P], dt.bfloat16,
                             kind="ExternalInput")
    w_in = nc.dram_tensor("w", [P, out_f], dt.bfloat16, kind="ExternalInput")
    bias_in = nc.dram_tensor("bias", [out_f, 1], dt.float32, kind="ExternalInput")
    dinv_in = nc.dram_tensor("dinv_sl", [1, cfg.nodes_per_core], dt.float32,
                             kind="ExternalInput")
    idx_in = nc.dram_tensor("idx_tab", [P, ICOLS], dt.int16,
                            kind="ExternalInput")
    d_in = nc.dram_tensor("d_tab", [P, DCOLS], dt.bfloat16, kind="ExternalInput")
    iota_in = nc.dram_tensor("iota", [P, G_MAX * P], dt.bfloat16,
                             kind="ExternalInput")
    ones_in = nc.dram_tensor("ones", [1, P], dt.float32, kind="ExternalInput")
    # partition-major self rows: self_tab[p, (blk*P + f)] = table[blk*P + p, f]
    self_in = nc.dram_tensor("self_tab", [P, cfg.bpc * P], dt.bfloat16,
                             kind="ExternalInput")
    out_t = nc.dram_tensor("out_t", [out_f, cfg.nodes_per_core], out_dt,
                           kind="ExternalOutput")

    with tile.TileContext(nc) as tc:
        with (
            tc.tile_pool(name="const", bufs=1) as const_pool,
            tc.tile_pool(name="idx", bufs=2) as idx_pool,
            tc.tile_pool(name="msg", bufs=2) as msg_pool,
            tc.tile_pool(name="sel", bufs=3) as sel_pool,
            tc.tile_pool(name="aggsb", bufs=3) as aggsb_pool,
            tc.tile_pool(name="tmp", bufs=3) as tmp_pool,
            tc.tile_pool(name="outsb", bufs=2) as out_pool,
            tc.tile_pool(name="psA", bufs=2, space="PSUM") as agg_psum,
            tc.tile_pool(name="psZ", bufs=2, space="PSUM") as z_psum,
            tc.tile_pool(name="psD", bufs=2, space="PSUM") as d_psum,
        ):
            w_sb = const_pool.tile([P, out_f], dt.bfloat16)
            nc.sync.dma_start(out=w_sb[:], in_=w_in[:])
            bias_sb = const_pool.tile([out_f, 1], dt.float32)
            nc.sync.dma_start(out=bias_sb[:], in_=bias_in[:])
            dinv_sb = const_pool.tile([1, cfg.nodes_per_core], dt.float32)
            nc.sync.dma_start(out=dinv_sb[:], in_=dinv_in[:])
            iota_sb = const_pool.tile([P, G_MAX * P], dt.bfloat16)
            nc.sync.dma_start(out=iota_sb[:], in_=iota_in[:])
            ones_sb = const_pool.tile([1, P], dt.float32)
            nc.sync.dma_start(out=ones_sb[:], in_=ones_in[:])
            d_sb = const_pool.tile([P, DCOLS], dt.bfloat16)
            nc.sync.dma_start(out=d_sb[:], in_=d_in[:])

            for si in range(NSB):
                icols = int(idx_cols[si])
                ioff = int(idx_off[si])
                idx_sb = idx_pool.tile([P, IDX_MAX], dt.int16)
                nc.sync.dma_start(out=idx_sb[:, :icols],
                                  in_=idx_in[:, ioff:ioff + icols])

                msg = msg_pool.tile([P, MSG_MAX, P], dt.bfloat16)
                # self rows: one fat DMA into cols 0..6
                nc.sync.dma_start(
                    out=msg[:, 0:SBW, :],
                    in_=self_in[:, si * SBW * P:(si + 1) * SBW * P]
                        .rearrange("p (b f) -> p b f", b=SBW))
                for ki in range(NK):
                    n_idx = int(sec_pad[si, ki])
                    mcol0 = int(sec_col0[si, ki])
                    icol0 = int(idx_col0[si, ki])
                    o0 = 0
                    while o0 < n_idx:
                        nn = min(MAX_CALL_IDX, n_idx - o0)
                        nc.gpsimd.dma_gather(
                            msg[:, mcol0 + o0 // P: mcol0 + (o0 + nn) // P, :],
                            src_tab[ki * cfg.src_window:
                                    (ki + 1) * cfg.src_window, :],
                            idx_sb[:, icol0 + o0 // 16:
                                   icol0 + (o0 + nn) // 16],
                            nn, nn, P,
                            queue_num=ki % 4,
                            single_packet=False,
                        )
                        o0 += nn

                out_sb = out_pool.tile([out_f, SBW * P], out_dt)
                for b in range(SBW):
                    b_loc = si * SBW + b
                    nd = int(ndcols[si, b])
                    dc0 = int(dcol_base[si, b])
                    sel = sel_pool.tile([P, G_MAX * P], dt.bfloat16)
                    nc.vector.tensor_tensor(
                        sel[:, :nd * P],
                        iota_sb[:, :nd * P],
                        d_sb[:, dc0:dc0 + nd].to_broadcast([P, nd, P]),
                        mybir.AluOpType.is_equal,
                    )

                    agg = agg_psum.tile([P, P], dt.float32)
                    ents = mm_sched[si][b]
                    for ei, (mcol, dci) in enumerate(ents):
                        nc.tensor.matmul(
                            agg[:],
                            lhsT=msg[:, mcol, :],
                            rhs=sel[:, dci * P:(dci + 1) * P],
                            start=(ei == 0), stop=(ei == len(ents) - 1),
                        )

                    # dinv broadcast tile (rank-1 matmul, ACT copy out)
                    dps = d_psum.tile([P, P], dt.float32)
                    nc.tensor.matmul(
                        dps[:], lhsT=ones_sb[:],
                        rhs=dinv_sb[:, b_loc * P:(b_loc + 1) * P],
                        start=True, stop=True)
                    dbc = aggsb_pool.tile([P, P], dt.float32, tag="dbc")
                    nc.scalar.copy(dbc[:], dps[:])

                    aggsb = aggsb_pool.tile([P, P], dt.bfloat16, tag="aggsb")
                    nc.scalar.copy(aggsb[:], agg[:])

                    zps = z_psum.tile([out_f, P], dt.float32)
                    nc.tensor.matmul(zps[:], lhsT=w_sb[:], rhs=aggsb[:],
                                     start=True, stop=True)

                    o_sl = out_sb[:, b * P:(b + 1) * P]
                    bias_bc = bias_sb[:, 0:1].to_broadcast([out_f, P])
                    if layer == 1:
                        t1 = tmp_pool.tile([P, P], dt.float32, tag="t1")
                        nc.vector.tensor_tensor(t1[:], zps[:], dbc[:],
                                                mybir.AluOpType.mult)
                        u = tmp_pool.tile([P, P], dt.float32, tag="u")
                        nc.vector.tensor_tensor(u[:], t1[:], bias_bc,
                                                mybir.AluOpType.add)
                        rr = tmp_pool.tile([P, P], dt.float32, tag="rr")
                        nc.vector.scalar_tensor_tensor(
                            rr[:], u[:], float(RRELU_SLOPE), u[:],
                            mybir.AluOpType.mult, mybir.AluOpType.max)
                        nc.vector.tensor_tensor(o_sl, rr[:], dbc[:],
                                                mybir.AluOpType.mult)
                    else:
                        t1 = tmp_pool.tile([out_f, P], dt.float32, tag="t1")
                        nc.vector.tensor_tensor(t1[:], zps[:], dbc[:out_f, :],
                                                mybir.AluOpType.mult)
                        nc.vector.tensor_tensor(o_sl, t1[:], bias_bc,
                                                mybir.AluOpType.add)

                nc.sync.dma_start(
                    out=out_t[:, si * SBW * P:(si + 1) * SBW * P],
                    in_=out_sb[:])

    nc.compile()
    return nc


# --------------------------------------------------------------------------
# orchestration
# --------------------------------------------------------------------------

def _iota_tile(G):
    return (np.tile(np.arange(P, dtype=np.float32), G)[None, :]
            .repeat(P, 0).astype(ml_dtypes.bfloat16))


def _self_tab(xs, cfg, c):
    """[128, bpc*128]: self_tab[p, blk*P + f] = xs[c*npc + blk*P + p, f]"""
    v = xs[c * cfg.nodes_per_core:(c + 1) * cfg.nodes_per_core]
    v = v.reshape(cfg.bpc, P, P).transpose(1, 0, 2).reshape(P, cfg.bpc * P)
    return np.ascontiguousarray(v)


def _run_gcn(x, edge_index, W1, b1, W2, b2, cfg, runner=None, want_times=False):
    meta = preprocess(np.asarray(edge_index), cfg)
    dinv = meta["dinv"]
    shared = meta["shared"]
    npc = cfg.nodes_per_core

    if runner is None:
        times = []

        def runner(nc, in_maps):
            r = run_bass_kernel_spmd(nc, in_maps, core_ids=list(range(cfg.n_cores)),
                                     trace=want_times)
            if want_times:
                times.append(r.exec_time_ns)
            return r.results
    else:
        times = None

    x = np.asarray(x, dtype=np.float32)
    xs = np.zeros((cfg.tab_rows, P), dtype=ml_dtypes.bfloat16)
    xs[:cfg.n_nodes] = (x * dinv[:cfg.n_nodes, None]).astype(ml_dtypes.bfloat16)

    iota = _iota_tile(shared["G_MAX"])
    ones = np.ones((1, P), np.float32)
    w1 = np.asarray(W1, np.float32).astype(ml_dtypes.bfloat16)
    w2 = np.asarray(W2, np.float32).astype(ml_dtypes.bfloat16)
    b1c = np.asarray(b1, np.float32).reshape(-1, 1)
    b2c = np.asarray(b2, np.float32).reshape(-1, 1)

    nc1 = build_layer_program(cfg, shared, layer=1)
    in_maps = [
        {"src_tab": xs, "w": w1, "bias": b1c, "iota": iota, "ones": ones,
         "self_tab": _self_tab(xs, cfg, c),
         **{kk: pc[kk] for kk in ("idx_tab", "d_tab", "dinv_sl")}}
        for c, pc in enumerate(meta["per_core"])
    ]
    res1 = runner(nc1, in_maps)

    gs = np.zeros((cfg.tab_rows, P), dtype=ml_dtypes.bfloat16)
    for c in range(cfg.n_cores):
        gs[c * npc:(c + 1) * npc] = res1[c]["out_t"].T

    nc2 = build_layer_program(cfg, shared, layer=2)
    for c in range(cfg.n_cores):
        in_maps[c] = dict(in_maps[c])
        in_maps[c]["src_tab"] = gs
        in_maps[c]["self_tab"] = _self_tab(gs, cfg, c)
        in_maps[c]["w"] = w2
        in_maps[c]["bias"] = b2c
    res2 = runner(nc2, in_maps)

    out = np.zeros((cfg.n_pad, cfg.out2_f), dtype=np.float32)
    for c in range(cfg.n_cores):
        out[c * npc:(c + 1) * npc] = res2[c]["out_t"].T
    out = out[:cfg.n_nodes]
    if want_times and times is not None:
        return out, times
    return out


def kernel(x, edge_index, W1, b1, W2, b2):
    return _run_gcn(x, edge_index, W1, b1, W2, b2, FULL)
